# revision 64
# baseline (speedup 1.0000x reference)
"""AlphaPermutationLayer Trainium2 kernel.

out[i, j] = sum_k softmax(alpha/T)[k] * (perm[k, i] == j),  N=2048, K=64.

Strategy: shard OUTPUT ROWS across the 8 cores (each output row i depends only
on perm[:, i] and alpha, so no collective is needed).  Per core (256 rows):
digit-split j = jq*64 + jf (jq in [0,32), jf in [0,64)); per row i
    out_i[jq, jf] = sum_k A_i[k, jq] * B_i[k, jf]
with A = onehot(perm>>6) (bf16, exact) stationary and B = onehot(perm&63)
scaled by alpha (hi/lo split for fp32-grade precision) moving. 256 tiny
TensorE matmuls per core accumulate straight into PSUM which holds the whole
2MB per-core output; ACT evacuates PSUM->SBUF; strided DMAs write DRAM.
"""

import os
import sys

sys.path.insert(0, "/opt/trn_rl_repo")

import numpy as np

N = 2048
K = 64
NCORES = 8
ROWS = N // NCORES          # 256 rows per core
DP = 32                     # stationary digit width (jq), psum partitions per row
DF = 64                     # moving digit width (jf), psum free per row
HILO = os.environ.get("KERNEL_HILO", "1") == "1"
F32 = os.environ.get("KERNEL_F32", "0") == "1"  # fp32 one-hots: 1 MM/pair, exact

LAST_EXEC_NS = None
LAST_RESULTS = None

_cached = {}


def _build_bass():
    import concourse.tile as tile
    from concourse import bacc, mybir
    from concourse.bass import _add_dep_helper

    fp32 = mybir.dt.float32
    bf16 = mybir.dt.bfloat16
    i16 = mybir.dt.int16

    nc = bacc.Bacc()

    ph_ext = nc.declare_dram_parameter("ph", [128, 128], i16, isOutput=False)
    al_ext = nc.declare_dram_parameter("alpha_t", [128, 1], fp32, isOutput=False)
    tp_ext = nc.declare_dram_parameter("temp_t", [128, 1], fp32, isOutput=False)
    ia_ext = nc.declare_dram_parameter("iota_a", [DP * 128], i16, isOutput=False)
    if_ext = nc.declare_dram_parameter("iota_f", [DF], i16, isOutput=False)
    px_ext = nc.declare_dram_parameter("plx", [128, 128 * DF], i16, isOutput=False)
    out_ext = nc.declare_dram_parameter("out", [ROWS, N], fp32, isOutput=True)

    with tile.TileContext(nc) as tc:
        with (
            tc.tile_pool(name="sbuf", bufs=1) as sb,
            tc.tile_pool(name="stage", bufs=4) as stp,
            tc.tile_pool(name="smax_psum", bufs=1, space="PSUM") as psmax,
            tc.tile_pool(name="psum", bufs=7, space="PSUM") as pp,
        ):
            # ---- load inputs -------------------------------------------------
            ph_t = sb.tile([128, 128], i16)
            al_t = sb.tile([128, 1], fp32)
            tp_t = sb.tile([128, 1], fp32)
            iota_a = sb.tile([128, DP, 128], i16)     # [p, c, i] = c
            iota_f = sb.tile([128, DF], i16)          # [p, f] = f
            plx_t = sb.tile([128, 128, DF], i16)      # [p, i, f] = pl[p, i]
            px_view = px_ext[:].rearrange("p (i f) -> p i f", f=DF)
            nc.sync.dma_start(out=plx_t[:, 0:32], in_=px_view[:, 0:32])
            nc.sync.dma_start(
                out=iota_f[:], in_=if_ext[:].partition_broadcast(128)
            )
            nc.sync.dma_start(out=ph_t[:], in_=ph_ext[:])
            nc.sync.dma_start(out=al_t[:], in_=al_ext[:])
            nc.sync.dma_start(out=tp_t[:], in_=tp_ext[:])
            nc.sync.dma_start(
                out=iota_a[:],
                in_=ia_ext[:].rearrange("(c i) -> c i", c=DP).partition_broadcast(128),
            )
            for g in range(1, 4):
                ih = slice(g * 32, g * 32 + 32)
                nc.sync.dma_start(out=plx_t[:, ih], in_=px_view[:, ih])

            # memsets first: dep-free, hide under the input DMAs. Only the
            # OFF-diagonal blocks of a_bd need zeroing (the diagonal blocks
            # are fully overwritten by the is_equal builds below).
            cdt = fp32 if F32 else bf16
            a_bd = sb.tile([128, 2 * DP, 128], cdt)
            nc.gpsimd.memset(a_bd[0:64, DP : 2 * DP, :], 0.0)
            nc.gpsimd.memset(a_bd[64:128, 0:DP, :], 0.0)

            # ---- softmax(alpha / T) -> per-partition scale(s) ---------------
            # partitions hold k twice (p = k + 64*h), so the partition-sum
            # double-counts; the *2 compensations are folded into the Exp bias
            # (exp(x + ln2) = 2 exp(x)) and the broadcast weights (2.0).
            rt_t = sb.tile([128, 1], fp32)
            e_t = sb.tile([128, 1], fp32)
            ln2_t = sb.tile([128, 1], fp32)
            prime_t = sb.tile([128, 1], fp32)
            nc.vector.memset(ln2_t[:], float(np.log(2.0)))
            # dep-free ACT op: hoists the one-time activation-table load off
            # the softmax critical path.
            nc.scalar.activation(
                out=prime_t[:], in_=ln2_t[:],
                func=mybir.ActivationFunctionType.Exp,
            )
            ones_col = sb.tile([128, 1], fp32)
            ones_row = sb.tile([1, 128], fp32)
            r_t = sb.tile([1, 1], fp32)
            alpha_f = sb.tile([128, 1], fp32)
            rec_rt_i = nc.vector.reciprocal(out=rt_t[:], in_=tp_t[:])
            # e = 2*exp(a*(1/T))  (scale fused into the activation)
            nc.scalar.activation(
                out=e_t[:],
                in_=al_t[:],
                func=mybir.ActivationFunctionType.Exp,
                bias=ln2_t[:],
                scale=rt_t[:],
            )
            nc.vector.memset(ones_col[:], 1.0)
            # e_t = 2*exp(s) (ln2 bias) and the partition-sum double-counts k
            # (both halves), so sum = 4S; broadcasting 2/sum gives 1/(2S) and
            # alpha = e_t * 1/(2S) = exp(s)/S — the true softmax.
            nc.vector.memset(ones_row[:], 2.0)
            sum_ps = psmax.tile([1, 1], fp32, tag="smax")
            # HAM pre-warm group 1: dep-free PE work before the softmax
            # matmuls (results overwritten by the real sum below).
            for _ in range(40):
                nc.tensor.matmul(
                    sum_ps[:], lhsT=ones_col[:], rhs=ones_col[:],
                    start=True, stop=True,
                )
            nc.tensor.matmul(sum_ps[:], lhsT=e_t[:], rhs=ones_col[:], start=True, stop=True)
            nc.vector.reciprocal(out=r_t[:], in_=sum_ps[:])
            rb_ps = psmax.tile([128, 1], fp32, tag="smax")
            nc.tensor.matmul(rb_ps[:], lhsT=ones_row[:], rhs=r_t[:], start=True, stop=True)
            # HAM pre-warm: keep the PE busy through the head-idle gap so the
            # clock gate reaches 8/8 before the real matmul stream begins.
            # Results go to the dead sum_ps cell (WAR on the reciprocal read
            # is ordered by Tile); sized to fit inside the idle gap.
            for _ in range(66):
                nc.tensor.matmul(
                    sum_ps[:], lhsT=ones_col[:], rhs=ones_col[:],
                    start=True, stop=True,
                )
            # alpha_f = e * (1/sum)  (fp32, true softmax values)
            alpha_last_i = nc.vector.tensor_tensor(
                out=alpha_f[:], in0=e_t[:], in1=rb_ps[:], op=mybir.AluOpType.mult
            )

            if HILO:
                ah_bf = sb.tile([128, 1], bf16)
                ah_f = sb.tile([128, 1], fp32)
                al_f = sb.tile([128, 1], fp32)
                nc.vector.tensor_copy(out=ah_bf[:], in_=alpha_f[:])
                nc.vector.tensor_copy(out=ah_f[:], in_=ah_bf[:])
                alpha_last_i = nc.vector.tensor_tensor(
                    out=al_f[:], in0=alpha_f[:], in1=ah_f[:],
                    op=mybir.AluOpType.subtract,
                )

            # ---- one-hot builds ---------------------------------------------
            # Row-PAIR scheme: pair column i couples rows (h=0, i) and
            # (h=1, i).  The stationary operand is block-diagonal over the
            # two k-halves so one matmul (contraction 128) computes both rows:
            #   A_bd[64h:64h+64, 32h:32h+32, i] = (ph[k,128h+i] == jq), 0 else
            #   B_t [p=(k,h), jf, i]            = (pl[k,128h+i] == jf)
            # out[64*pp+32*h+jq, s] psum, pair i = b*16 + pp*8 + s.
            Copy = mybir.ActivationFunctionType.Copy
            b_t = sb.tile([128, 128, DF], cdt)      # i-major: rhs reads dense
            if HILO and not F32:
                bs_hi = sb.tile([128, 128, DF], cdt)
                bs_lo = sb.tile([128, 128, DF], cdt)
                b_list = [bs_hi, bs_lo]
            else:
                bs_hi = sb.tile([128, 128, DF], cdt)
                b_list = [bs_hi]

            NCHUNK = 4
            CW = 128 // NCHUNK
            for g in range(NCHUNK):
                ic = slice(g * CW, (g + 1) * CW)
                for h in range(2):
                    kp = slice(64 * h, 64 * h + 64)
                    a_i = nc.vector.tensor_tensor(
                        out=a_bd[kp, DP * h : DP * h + DP, ic],
                        in0=ph_t[kp, ic].unsqueeze(1).to_broadcast([64, DP, CW]),
                        in1=iota_a[kp, :, ic],
                        op=mybir.AluOpType.is_equal,
                    )
                    if g == 0 and h == 0:
                        # order the alpha-chain reciprocal before any DVE
                        # build work (scheduler hint, no semaphore)
                        _add_dep_helper(a_i.ins, rec_rt_i.ins, sync=False,
                                        reason="alpha recip before builds")
                    if g == 1 and h == 0:
                        # alpha-chain DVE ops run before chunk-1 builds
                        _add_dep_helper(a_i.ins, alpha_last_i.ins, sync=False,
                                        reason="alpha chain before chunk1")
                # B build i-major (dense rhs for the matmul): both operands
                # have inner step 1 (plx is host-expanded) -> 2x mode.
                nc.vector.tensor_tensor(
                    out=b_t[:, ic, :],
                    in0=plx_t[:, ic, :],
                    in1=iota_f[:].unsqueeze(1).to_broadcast([128, CW, DF]),
                    op=mybir.AluOpType.is_equal,
                )
                # scale passes on DVE (4x tensor_scalar)
                nc.vector.tensor_scalar(
                    out=bs_hi[:, ic, :],
                    in0=b_t[:, ic, :],
                    scalar1=(ah_f[:] if (HILO and not F32) else alpha_f[:]),
                    scalar2=None,
                    op0=mybir.AluOpType.mult,
                )
                if HILO and not F32:
                    nc.vector.tensor_scalar(
                        out=bs_lo[:, ic, :],
                        in0=b_t[:, ic, :],
                        scalar1=al_f[:],
                        scalar2=None,
                        op0=mybir.AluOpType.mult,
                    )

            # ---- per-pair matmuls + evacuation + store ----------------------
            oview = out_ext[:].rearrange(
                "(hh bb pp s) (q f) -> bb pp hh q s f", hh=2, bb=8, pp=2, s=8,
                q=DP, f=DF,
            )
            for b in range(8):
                bank = pp.tile([128, 8, DF], mybir.dt.float32, tag="bank")
                # s outer / pi inner: consecutive matmuls alternate array
                # col-groups so LDWEIGHTS of the next pair overlaps the
                # in-flight matmul (same col-group back-to-back serializes).
                for s in range(8):
                    for pi in range(2):
                        i = b * 16 + pi * 8 + s
                        out_ap = bank[64 * pi : 64 * pi + 64, s]
                        for m, bsrc in enumerate(b_list):
                            nc.tensor.matmul(
                                out_ap,
                                lhsT=a_bd[:, :, i],
                                rhs=bsrc[:, i, :],
                                start=(m == 0),
                                stop=(m == len(b_list) - 1),
                                tile_position=(0, 64 * pi),
                            )
                stage = stp.tile([128, 8, DF], mybir.dt.float32, tag="stage")
                if b < 6:
                    nc.scalar.activation(out=stage[:], in_=bank[:], func=Copy)
                else:
                    nc.vector.tensor_copy(out=stage[:], in_=bank[:])
                # psum partition q' = 64*pp + 32*h + jq ; row = 128h+16b+8pp+s
                for pi in range(2):
                    for h in range(2):
                        eng = nc.sync if (pi + h) % 2 == 0 else nc.scalar
                        eng.dma_start(
                            out=oview[b, pi, h],
                            in_=stage[64 * pi + 32 * h : 64 * pi + 32 * h + 32],
                        )
    if not nc.is_finalized():
        nc.finalize()
    return nc


def _prep_inputs(alpha_weights, perm_vectors, temperature):
    a = np.asarray(alpha_weights, dtype=np.float32).reshape(K)
    T = np.asarray(temperature, dtype=np.float32).reshape(())
    perm = np.asarray(perm_vectors).astype(np.int64).reshape(K, N)
    ph = (perm >> 6).astype(np.int16)
    pl = (perm & 63).astype(np.int16)
    al_t = np.concatenate([a, a])[:, None].copy()          # [128, 1]
    tp_t = np.full((128, 1), T, dtype=np.float32)
    iota_a = np.repeat(np.arange(DP), 128).astype(np.int16)
    iota_f = np.arange(DF).astype(np.int16)
    in_maps = []
    for c in range(NCORES):
        phc = ph[:, c * ROWS : (c + 1) * ROWS].reshape(K, 2, 128)
        plc = pl[:, c * ROWS : (c + 1) * ROWS].reshape(K, 2, 128)
        plc = plc.transpose(1, 0, 2).reshape(128, 128)
        plx = np.broadcast_to(plc[:, :, None], (128, 128, DF))
        in_maps.append(
            {
                "ph": phc.transpose(1, 0, 2).reshape(128, 128).copy(),
                "plx": plx.reshape(128, 128 * DF).copy(),
                "alpha_t": al_t,
                "temp_t": tp_t,
                "iota_a": iota_a,
                "iota_f": iota_f,
            }
        )
    return in_maps


def _install_ntff_hook():
    """Provide antenv.axon_hooks (missing in this image) so that
    run_bass_kernel_spmd(trace=True) can capture NTFF profiles via the
    axon PJRT .so (same mechanism as trn_agent_boot.trn_boot)."""
    import contextlib
    import ctypes
    import types

    try:
        from antenv.axon_hooks import get_axon_ntff_profile_hook  # noqa: F401

        return True
    except ImportError:
        pass
    so_path = "/opt/axon/libaxon_pjrt.so"
    if not os.path.exists(so_path):
        return False
    lib = ctypes.CDLL(so_path)
    if not hasattr(lib, "axon_start_nrt_profile"):
        return False
    lib.axon_start_nrt_profile.argtypes = [
        ctypes.POINTER(ctypes.c_int64),
        ctypes.c_size_t,
    ]
    lib.axon_start_nrt_profile.restype = ctypes.c_int64
    lib.axon_stop_nrt_profile.argtypes = [ctypes.c_char_p]
    lib.axon_stop_nrt_profile.restype = ctypes.c_int64

    @contextlib.contextmanager
    def _hook(output_dir, device_ids):
        import jax

        jax.devices()
        if device_ids:
            ids = (ctypes.c_int64 * len(device_ids))(*device_ids)
            rc = lib.axon_start_nrt_profile(ids, len(device_ids))
        else:
            rc = lib.axon_start_nrt_profile(None, 0)
        if rc != 0:
            raise RuntimeError(f"axon_start_nrt_profile rc={rc}")
        try:
            yield
        finally:
            n = lib.axon_stop_nrt_profile(str(output_dir).encode())
            print(f"ntff profile: {n} file(s) written to {output_dir}")

    import antenv

    mod = types.ModuleType("antenv.axon_hooks")
    mod.get_axon_ntff_profile_hook = lambda: _hook
    mod.set_axon_ntff_profile_hook = lambda h: None
    sys.modules["antenv.axon_hooks"] = mod
    antenv.axon_hooks = mod
    return True


def kernel(alpha_weights, perm_vectors, temperature):
    global LAST_EXEC_NS, LAST_RESULTS
    from concourse.bass_utils import run_bass_kernel_spmd

    if "nc" not in _cached:
        _cached["nc"] = _build_bass()
    nc = _cached["nc"]
    in_maps = _prep_inputs(alpha_weights, perm_vectors, temperature)
    core_ids = list(range(NCORES))
    trace = os.environ.get("KERNEL_TRACE", "0") == "1"
    if trace:
        trace = _install_ntff_hook()
    try:
        res = run_bass_kernel_spmd(nc, in_maps, core_ids, trace=trace)
    except Exception:
        if not trace:
            raise
        res = run_bass_kernel_spmd(nc, in_maps, core_ids, trace=False)
    LAST_EXEC_NS = res.exec_time_ns
    LAST_RESULTS = res
    out = np.concatenate([res.results[c]["out"] for c in range(NCORES)], axis=0)
    return out.astype(np.float32)


if __name__ == "__main__":
    rng = np.random.default_rng(0)
    a = rng.standard_normal(K).astype(np.float32)
    perm = np.stack([rng.permutation(N) for _ in range(K)]).astype(np.int64)
    T = np.ones((), np.float32)
    out = kernel(a, perm, T)
    # numpy reference
    al = np.exp(a / T - (a / T).max())
    al /= al.sum()
    exp = np.zeros((N, N), np.float32)
    np.add.at(exp, (np.broadcast_to(np.arange(N), (K, N)), perm), al[:, None])
    print("max abs err:", np.abs(out - exp).max(), "max ref:", np.abs(exp).max())
    print("exec ns:", LAST_EXEC_NS)
